# revision 59
# baseline (speedup 1.0000x reference)
"""Multi-head attention (B=4, S=2048, D=1024, H=16, causal + RoPE) on 8 trn2 cores.

Sharding: head-parallel tensor parallelism. Core c owns heads {2c, 2c+1}:
  - Q/K/V projections for its 2 heads over all B*S positions (bf16 matmuls),
  - RoPE + causal softmax attention,
  - row-parallel partial out-projection; host sums the 8 bf16 partials in f32.

Key layout tricks vs the fp32 baseline:
  - every matmul operand is bf16 (1 PE cycle/row instead of fp32's 4),
  - head-dim rows are stored interleaved (pair (d, d+32) -> rows (2d, 2d+1)) so
    RoPE's rotate-half is a single DVE stream_shuffle (even<->odd partitions),
    with the sign folded into the sin table; scores are invariant to the
    permutation because q and k use the same order,
  - V is projected straight into natural [pos, hd] layout (stationary x tile),
    so no PE transposes,
  - causal masking: off-diagonal key chunks are skipped entirely, diagonal
    chunks compute only the live q-range and multiply a 128-wide triangle mask;
    persistent zero-prefixed ex tiles keep the AV matmul full width,
  - exp is batched two key-chunks per Activation instruction (PSUM pair tiles),
  - softmax denominator via an extra ones column in the AV stationary operand.
"""

import contextlib
import os
import sys

import numpy as np

# the bass/concourse runtime lives here in the execution environment
if os.path.isdir("/opt/trn_rl_repo") and "/opt/trn_rl_repo" not in sys.path:
    sys.path.insert(0, "/opt/trn_rl_repo")

# ---- fixed problem config ----
B, S, D = 4, 2048, 1024
H, HD = 16, 64
NCORES = 8
HPC = H // NCORES          # heads per core = 2
ROPE_BASE = 10000.0

QBLK = 512                 # query block (free dim of scores/AV matmuls)
JCH = 128                  # key chunk (partition dim of scores^T)


# --------------------------------------------------------------------------
# host-side helpers
# --------------------------------------------------------------------------

def _bf16():
    import ml_dtypes
    return ml_dtypes.bfloat16


def _head_perm():
    """Permutation of the 64 head dims: pair (j, j+32) -> rows (2j, 2j+1)."""
    p = np.empty(HD, dtype=np.int64)
    for j in range(HD // 2):
        p[2 * j] = j
        p[2 * j + 1] = j + HD // 2
    return p


def _rope_tables_T(s):
    """cos/sin tables in permuted-interleaved row order, [HPC*64, s].

    Row 2j and 2j+1 share frequency j; sin carries the rotate-half sign:
    row 2j gets -sin, row 2j+1 gets +sin.
    """
    inv_freq = 1.0 / (ROPE_BASE ** (np.arange(0, HD, 2, dtype=np.float32) / np.float32(HD)))
    t = np.arange(s, dtype=np.float32)
    freqs = np.outer(inv_freq, t).astype(np.float32)          # [32, s]
    cos = np.cos(freqs)
    sin = np.sin(freqs)
    cos_i = np.repeat(cos, 2, axis=0)                          # [64, s]
    sin_i = np.empty_like(cos_i)
    sin_i[0::2] = -sin
    sin_i[1::2] = sin
    bf16 = _bf16()
    return (
        np.tile(cos_i, (HPC, 1)).astype(bf16).copy(),
        np.tile(sin_i, (HPC, 1)).astype(bf16).copy(),
    )


def _tri_mask():
    """tri[j, u] = 1.0 iff u >= j  (keep mask on a diagonal 128x128 strip)."""
    j = np.arange(JCH)[:, None]
    u = np.arange(JCH)[None, :]
    return (u >= j).astype(_bf16())


def _key_bias(attention_mask, s):
    """[128, B*(s//128)] additive bias per key position: 0 valid, -30000 pad."""
    b = attention_mask.shape[0]
    kb = np.where(np.asarray(attention_mask) == 0, np.float32(-30000.0), np.float32(0.0))
    kb = kb.reshape(b * (s // JCH), JCH).T.astype(np.float32)
    return np.ascontiguousarray(kb)


# --------------------------------------------------------------------------
# device program
# --------------------------------------------------------------------------

def emit(tc, outs, ins, *, b, s, d, use_kb):
    """Emit the per-core program into TileContext tc.

    ins: xT [ngroups, 128, kchunks, 512] bf16 (group-blocked x^T),
         wq/wk/wv [d, 128] bf16 (wq/wk rope-permuted), wo [128, d] bf16,
         cosT/sinT [128, s] bf16, tri [128, 128] bf16, keybias [128, b*s/128] f32.
    outs: yD [b, s/512, d/256, 128, 2, 512] bf16 partial (host reassembles+sums).
    use_kb: apply per-key-chunk additive bias inside exp (general mask path;
            forces one exp per key chunk instead of per pair).
    """
    import concourse.bass as bass
    import concourse.mybir as mybir

    nc = tc.nc
    f32 = mybir.dt.float32
    bf16 = mybir.dt.bfloat16
    AF = mybir.ActivationFunctionType

    bs = b * s
    kchunks = d // 128          # contraction chunks for projections = 8
    ngroups = bs // QBLK        # 512-wide position groups = 16
    nt = bs // JCH              # 128-wide position chunks = 64
    nqb = s // QBLK             # query blocks per sequence = 4
    njd = QBLK // JCH           # diagonal chunks per query block = 4
    nnch = d // 128             # out-proj n chunks = 8
    scale = float(1.0 / np.sqrt(HD))
    shuf_mask = [i ^ 1 for i in range(32)]   # even<->odd partition swap

    xT, wq, wk, wv, wo = ins["xT"], ins["wq"], ins["wk"], ins["wv"], ins["wo"]
    cosT, sinT, tri, keybias = ins["cosT"], ins["sinT"], ins["tri"], ins["keybias"]
    yD = outs["yD"]

    ctx = contextlib.ExitStack()
    with ctx:
        singles = ctx.enter_context(tc.tile_pool(name="singles", bufs=1))
        xpool = ctx.enter_context(tc.tile_pool(name="xtiles", bufs=4))
        # PSUM budget (8 banks of 2KB/partition):
        #   'big'  [128,2,512] f32 = 2 banks x 2 bufs = 4   (score pairs + y pairs)
        #   'proj' [128,512]   f32 = 1 bank  x 2 bufs = 2   (qkv projections)
        #   'acc'  [128,512]   f32 = 1 bank  x 2 bufs = 2   (AV accumulator)
        ps_big = ctx.enter_context(tc.tile_pool(name="psbig", bufs=2, space="PSUM"))
        ps_proj = ctx.enter_context(tc.tile_pool(name="psproj", bufs=2, space="PSUM"))
        ps_acc = ctx.enter_context(tc.tile_pool(name="psacc", bufs=2, space="PSUM"))
        rawpool = ctx.enter_context(tc.tile_pool(name="raw", bufs=2))
        tmppool = ctx.enter_context(tc.tile_pool(name="tmp", bufs=2))
        expool = ctx.enter_context(tc.tile_pool(name="ex", bufs=4))
        outhpool = ctx.enter_context(tc.tile_pool(name="outh", bufs=2))
        oh1pool = ctx.enter_context(tc.tile_pool(name="oh1", bufs=2))
        recpool = ctx.enter_context(tc.tile_pool(name="rec", bufs=2))
        bctpool = ctx.enter_context(tc.tile_pool(name="bct", bufs=2))
        ypool = ctx.enter_context(tc.tile_pool(name="yev", bufs=3))

        # ---- persistent SBUF state ----
        wq_sb = singles.tile([128, kchunks, 128], bf16)
        wk_sb = singles.tile([128, kchunks, 128], bf16)
        wv_sb = singles.tile([128, kchunks, 128], bf16)
        wo_sb = singles.tile([128, nnch, 128], bf16)
        cos_sb = singles.tile([128, s], bf16)
        sin_sb = singles.tile([128, s], bf16)
        tri_sb = singles.tile([128, JCH], bf16)
        kb_sb = singles.tile([128, nt], f32)
        qT_sb = singles.tile([128, bs], bf16)
        kT_sb = singles.tile([128, bs], bf16)
        # V natural layout + ones column: [pos chunk, head, 64 data + 1 ones + pad]
        v_sb = singles.tile([128, nt, 2, 66], bf16)
        # persistent ex pair-tiles for diagonal chunks r=(2p, 2p+1)
        exdp = [singles.tile([128, 2, QBLK], bf16, name=f"exdp{p}") for p in range(njd // 2)]

        # ---- phase A: projection groups, split into schedulable quanta ----
        xt_map = {}
        xt_refs = {}

        def xt_load(g, split=False):
            if g in xt_map:
                return
            t = xpool.tile([128, kchunks, QBLK], bf16, tag="xt", name="xt")
            if split:  # per-chunk DMAs: first matmul starts after 1/8 arrives
                for kc in range(kchunks):
                    nc.sync.dma_start(out=t[:, kc, :], in_=xT[g, :, kc, :])
            else:
                nc.sync.dma_start(out=t[:, :, :], in_=xT[g, :, :, :])
            xt_map[g] = t
            xt_refs[g] = 6  # quanta still holding this tile

        # startup order: q/k weights and the first x tile first, so the PE can
        # start within a few us; everything else after (weights arrive in
        # device layout [p, chunk, m] already — flat contiguous DMAs)
        nc.sync.dma_start(out=wq_sb[:, :, :], in_=wq[:, :, :])
        nc.sync.dma_start(out=wk_sb[:, :, :], in_=wk[:, :, :])
        xt_load(0)
        nc.sync.dma_start(out=wv_sb[:, :, :], in_=wv[:, :, :])
        nc.sync.dma_start(out=cos_sb[:, :], in_=cosT[:, :])
        nc.sync.dma_start(out=sin_sb[:, :], in_=sinT[:, :])
        nc.sync.dma_start(out=tri_sb[:, :], in_=tri[:, :])
        nc.sync.dma_start(out=wo_sb[:, :, :], in_=wo[:, :, :])
        if use_kb:
            nc.sync.dma_start(out=kb_sb[:, :], in_=keybias[:, :])
        nc.vector.memset(v_sb[:, :, :, 64:65], 1.0)

        def xt_done(g):
            xt_refs[g] -= 1
            if xt_refs[g] == 0:
                del xt_map[g]

        def quantum_qk(g, which):
            g0 = g * QBLK
            ps0 = g0 % s
            xt = xt_map[g]
            w_sb, dst = (wq_sb, qT_sb) if which == 0 else (wk_sb, kT_sb)
            pp = ps_proj.tile([128, QBLK], f32, tag="proj", name="pp")
            for kc in range(kchunks):
                nc.tensor.matmul(
                    pp[:, :],
                    w_sb[:, kc, :],
                    xt[:, kc, :],
                    start=(kc == 0),
                    stop=(kc == kchunks - 1),
                )
            # RoPE: dst = raw*cos + shuffle(raw)*sin_signed
            raw = rawpool.tile([128, QBLK], bf16, tag="raw", name="raw")
            if g >= 10:
                nc.vector.tensor_copy(raw[:, :], pp[:, :])
            else:
                nc.scalar.copy(out=raw[:, :], in_=pp[:, :])
            tmp = tmppool.tile([128, QBLK], bf16, tag="rope_tmp", name="tmp")
            nc.vector.stream_shuffle(tmp[:, :], raw[:, :], shuf_mask)
            nc.vector.tensor_mul(tmp[:, :], tmp[:, :], sin_sb[:, ps0 : ps0 + QBLK])
            nc.vector.tensor_mul(dst[:, g0 : g0 + QBLK], raw[:, :], cos_sb[:, ps0 : ps0 + QBLK])
            nc.vector.tensor_add(dst[:, g0 : g0 + QBLK], dst[:, g0 : g0 + QBLK], tmp[:, :])
            xt_done(g)

        def quantum_v(g, c):
            xt = xt_map[g]
            t = g * (QBLK // JCH) + c
            pv = ps_proj.tile([128, QBLK], f32, tag="proj", name="pv")
            for kc in range(kchunks):
                nc.tensor.matmul(
                    pv[:, 0:128],
                    xt[:, kc, c * JCH : (c + 1) * JCH],
                    wv_sb[:, kc, :],
                    start=(kc == 0),
                    stop=(kc == kchunks - 1),
                )
            nc.vector.tensor_copy(
                v_sb[:, t, :, 0:64],
                pv[:, 0:128].rearrange("p (h x) -> p h x", h=2),
            )
            xt_done(g)

        def group_quanta(g):
            yield lambda: quantum_qk(g, 0)
            yield lambda: quantum_qk(g, 1)
            for c in range(QBLK // JCH):
                yield lambda c=c: quantum_v(g, c)

        # ---- phases C+D: attention + out-projection, per (batch, query block) ----
        # Software pipelining: (a) AV matmuls lag one score-pair behind, so the
        # PE never sits waiting on the exp of the pair it just computed;
        # (b) the out-projection of iteration t is emitted inside iteration
        # t+1's exp windows; (c) later batches' projection groups are
        # interleaved between attention units as exp-independent PE filler.

        def emit_outproj(outh, bi, qb):
            for t2 in range(nnch // 2):
                py = ps_big.tile([128, 2, QBLK], f32, tag="big", name="py")
                for i in range(2):
                    nc.tensor.matmul(
                        py[:, i, :], wo_sb[:, 2 * t2 + i, :], outh[:, :],
                        start=True, stop=True,
                    )
                ysb = ypool.tile([128, 2, QBLK], bf16, tag="yevac", name="ysb")
                # alternate evac engines so neither gates the py pipeline
                if t2 % 2 == 0:
                    nc.scalar.copy(out=ysb[:, :, :], in_=py[:, :, :])
                else:
                    nc.vector.tensor_copy(ysb[:, :, :], py[:, :, :])
                nc.sync.dma_start(out=yD[bi, qb, t2, :, :, :], in_=ysb[:, :, :])

        deferred = [None]

        def emit_cunit(bi, qb, h, outh, filler):
                    q0 = bi * s + qb * QBLK
                    hb = h * 64
                    nj = njd * (qb + 1)
                    pav = ps_acc.tile([128, QBLK], f32, tag="acc", name="pav")
                    pends = []  # (exs, js) of up to 2 previous score pairs
                    for pi in range(nj // 2):
                        pscore = ps_big.tile([128, 2, QBLK], f32, tag="big", name="pscore")
                        js = [pi * 2, pi * 2 + 1]
                        exs = []
                        for i, jc in enumerate(js):
                            jg = bi * s + jc * JCH
                            r = jc - njd * qb          # >=0 on diagonal chunks
                            lo = r * JCH if r >= 0 else 0
                            nc.tensor.matmul(
                                pscore[:, i, lo:QBLK],
                                kT_sb[hb : hb + 64, jg : jg + JCH],
                                qT_sb[hb : hb + 64, q0 + lo : q0 + QBLK],
                                start=True,
                                stop=True,
                            )
                        if use_kb:
                            # general path: per-chunk exp with key bias column
                            for i, jc in enumerate(js):
                                r = jc - njd * qb
                                lo = r * JCH if r >= 0 else 0
                                if r >= 0:
                                    ex = exdp[r // 2][:, r % 2, :]
                                else:
                                    ex = expool.tile([128, 2, QBLK], bf16, tag="ex", name="ex")[:, i, :]
                                kbi = bi * (s // JCH) + jc
                                nc.scalar.activation(
                                    out=ex[:, lo:QBLK], in_=pscore[:, i, lo:QBLK],
                                    func=AF.Exp, bias=kb_sb[:, kbi : kbi + 1], scale=scale,
                                )
                                exs.append(ex)
                        elif js[1] - njd * qb >= 0:
                            # pair contains diagonal chunks: exp each sliced
                            for i, jc in enumerate(js):
                                r = jc - njd * qb
                                assert r >= 0
                                ex = exdp[r // 2][:, r % 2, :]
                                nc.scalar.activation(
                                    out=ex[:, r * JCH : QBLK],
                                    in_=pscore[:, i, r * JCH : QBLK],
                                    func=AF.Exp, scale=scale,
                                )
                                exs.append(ex)
                        else:
                            # both off-diagonal: one batched exp
                            expair = expool.tile([128, 2, QBLK], bf16, tag="ex", name="ex")
                            nc.scalar.activation(
                                out=expair[:, :, :], in_=pscore[:, :, :],
                                func=AF.Exp, scale=scale,
                            )
                            exs = [expair[:, 0, :], expair[:, 1, :]]
                        # triangle mask on the diagonal 128-wide strip
                        for i, jc in enumerate(js):
                            r = jc - njd * qb
                            if r >= 0:
                                sl = slice(r * JCH, (r + 1) * JCH)
                                nc.vector.tensor_mul(exs[i][:, sl], exs[i][:, sl], tri_sb[:, :])

                        def emit_avs(avlist):
                            for ex_ap, jc in avlist:
                                t = bi * (s // JCH) + jc
                                r = jc - njd * qb
                                lo = r * JCH if r > 0 else 0
                                # diagonal AVs write a shrinking column range;
                                # each closes its range (stop is sim-side
                                # bookkeeping only, HW accumulates per element)
                                nc.tensor.matmul(
                                    pav[0:65, lo:QBLK], v_sb[:, t, h, 0:65], ex_ap[:, lo:QBLK],
                                    start=(jc == 0), stop=(r >= 0),
                                    skip_group_check=(r > 0),
                                )

                        filler()  # PE-filler quantum inside the exp window
                        if len(pends) >= 2:
                            emit_avs(pends.pop(0))
                        pends.append(list(zip(exs, js)))
                    for p_ in pends:
                        emit_avs(p_)
                    if h == 1 and deferred[0] is not None:
                        emit_outproj(*deferred[0])
                        deferred[0] = None
                    # normalize: rows /= denominator row (pav row 64)
                    rec = recpool.tile([128, QBLK], f32, tag="rec")
                    nc.vector.reciprocal(rec[64:65, :], pav[64:65, :])
                    row = rec[64:65, :]
                    bc3 = bass.AP(tensor=row.tensor, offset=row.offset,
                                  ap=[list(row.ap[0])] + [[0, 64]] + [list(row.ap[1])])
                    bct = bctpool.tile([64, QBLK], f32, tag="bct")
                    nc.sync.dma_start(out=bct[0:64, :], in_=bc3)
                    if h == 0:
                        nc.vector.tensor_mul(outh[0:64, :], pav[0:64, :], bct[0:64, :])
                    else:
                        oh1 = oh1pool.tile([64, QBLK], bf16, tag="oh1", name="oh1")
                        nc.vector.tensor_mul(oh1[0:64, :], pav[0:64, :], bct[0:64, :])
                        nc.sync.dma_start(out=outh[64:128, :], in_=oh1[0:64, :])

        # driver: units run qb-major — (qb, bi, h) — so the group for
        # (batch gb, quarter gq), first read by unit 8*gq + 2*gb, becomes due
        # every other unit across the whole run: even PE filler everywhere.
        # Projection quanta are queued with that deadline and pulled one per
        # score-pair inside the exp windows; anything still pending at its
        # deadline is flushed before the unit that needs it.
        from collections import deque

        pending = deque()  # (deadline_unit, closure)
        enq = {}
        for gb in range(b):
            for gq in range(nqb):
                g = njd * gb + gq
                deadline = 8 * gq + 2 * gb
                enq.setdefault(max(0, deadline - 4), []).append((deadline, g))

        def filler():
            if pending:
                pending.popleft()[1]()

        unit = 0
        outh = None
        for qb in range(nqb):
            for bi in range(b):
                for h in range(2):
                    if h == 0:
                        outh = outhpool.tile([128, QBLK], bf16, tag="outh", name="outh")
                    for deadline, g in enq.get(unit, []):
                        xt_load(g)
                        pending.extend((deadline, q) for q in group_quanta(g))
                    while pending and pending[0][0] <= unit:
                        pending.popleft()[1]()
                    emit_cunit(bi, qb, h, outh, filler)
                    if h == 1:
                        deferred[0] = (outh, bi, qb)
                    unit += 1
        while pending:
            pending.popleft()[1]()
        emit_outproj(*deferred[0])


# --------------------------------------------------------------------------
# host entry point
# --------------------------------------------------------------------------

def _assemble_y(arr, b, s, d):
    """[b, s/512, d/256, 128, 2, 512] partial -> [d, b*s] f32."""
    a = np.asarray(arr).astype(np.float32)
    # rows = (t2, i, p), cols = (bi, qb, q)
    return np.transpose(a, (2, 4, 3, 0, 1, 5)).reshape(d, b * s)


def _shard_inputs(x, attention_mask, w_qkv, w_out, b, s, d):
    """Build the per-core input maps (host-side shard/cast/permute)."""
    bf16 = _bf16()
    xT = np.ascontiguousarray(
        np.asarray(x, dtype=np.float32).reshape(b * s, d).T
    )
    # group-blocked layout: [group, p, kc, q]
    xT = np.ascontiguousarray(
        xT.reshape(d // 128, 128, (b * s) // QBLK, QBLK).transpose(2, 1, 0, 3)
    ).astype(bf16)
    w_qkv = np.asarray(w_qkv, dtype=np.float32)
    w_out = np.asarray(w_out, dtype=np.float32)
    cosT, sinT = _rope_tables_T(s)
    tri = _tri_mask()
    keybias = _key_bias(attention_mask, s)
    perm = _head_perm()
    cw = HPC * HD  # 128 columns per core
    # rope-permute q/k head dims (per 64-wide head block)
    qp = np.concatenate([h * HD + perm for h in range(H)])
    wq_p = w_qkv[:, 0 * d : 1 * d][:, qp].astype(bf16)
    wk_p = w_qkv[:, 1 * d : 2 * d][:, qp].astype(bf16)
    wv_f = w_qkv[:, 2 * d : 3 * d].astype(bf16)
    wo_f = w_out.astype(bf16)
    def wlayout(w):  # [d, 128] -> [128 p, d/128 kc, 128 m]
        return np.ascontiguousarray(w.reshape(d // 128, 128, 128).transpose(1, 0, 2))

    in_maps = []
    for c in range(NCORES):
        sl = slice(c * cw, (c + 1) * cw)
        in_maps.append(
            {
                "xT": xT,
                "wq": wlayout(wq_p[:, sl]),
                "wk": wlayout(wk_p[:, sl]),
                "wv": wlayout(wv_f[:, sl]),
                "wo": np.ascontiguousarray(wo_f[sl, :].reshape(128, d // 128, 128)),
                "cosT": cosT,
                "sinT": sinT,
                "tri": tri,
                "keybias": keybias,
            }
        )
    return in_maps


_PROG_CACHE = {}


def _build_program(b, s, d, variant="fast"):
    use_kb = variant == "pad"
    key = (b, s, d, use_kb)
    if key in _PROG_CACHE:
        return _PROG_CACHE[key]
    import concourse.mybir as mybir
    from concourse import bacc
    from concourse.tile import TileContext

    f32 = mybir.dt.float32
    bf16 = mybir.dt.bfloat16
    nc = bacc.Bacc("TRN2", target_bir_lowering=False, debug=False)
    bs = b * s
    ins = {
        "xT": nc.dram_tensor(
            "xT", [bs // QBLK, 128, d // 128, QBLK], bf16, kind="ExternalInput"
        ).ap(),
        "wq": nc.dram_tensor("wq", [128, d // 128, 128], bf16, kind="ExternalInput").ap(),
        "wk": nc.dram_tensor("wk", [128, d // 128, 128], bf16, kind="ExternalInput").ap(),
        "wv": nc.dram_tensor("wv", [128, d // 128, 128], bf16, kind="ExternalInput").ap(),
        "wo": nc.dram_tensor("wo", [128, d // 128, 128], bf16, kind="ExternalInput").ap(),
        "cosT": nc.dram_tensor("cosT", [128, s], bf16, kind="ExternalInput").ap(),
        "sinT": nc.dram_tensor("sinT", [128, s], bf16, kind="ExternalInput").ap(),
        "tri": nc.dram_tensor("tri", [128, JCH], bf16, kind="ExternalInput").ap(),
        "keybias": nc.dram_tensor("keybias", [128, bs // JCH], f32, kind="ExternalInput").ap(),
    }
    outs = {
        "yD": nc.dram_tensor(
            "yD", [b, s // QBLK, d // 256, 128, 2, QBLK], bf16, kind="ExternalOutput"
        ).ap()
    }
    with TileContext(nc) as tc:
        emit(tc, outs, ins, b=b, s=s, d=d, use_kb=use_kb)
    nc.compile()
    _PROG_CACHE[key] = nc
    return nc


def kernel(x, attention_mask, w_qkv, w_out, *, variant=None, trace=False):
    from concourse import bass_utils

    b, s, d = x.shape
    if variant is None:
        variant = "pad" if np.any(np.asarray(attention_mask) == 0) else "fast"
    nc = _build_program(b, s, d, variant)
    in_maps = _shard_inputs(x, attention_mask, w_qkv, w_out, b, s, d)
    res = bass_utils.run_bass_kernel_spmd(
        nc, in_maps, core_ids=list(range(NCORES)), trace=trace
    )
    acc = _assemble_y(res.results[0]["yD"], b, s, d)
    for c in range(1, NCORES):
        acc = acc + _assemble_y(res.results[c]["yD"], b, s, d)
    out = np.ascontiguousarray(acc.T).reshape(b, s, d).astype(np.float32)
    if trace:
        return out, res
    return out
